# revision 15
# baseline (speedup 1.0000x reference)
"""Trainium2 Bass kernel for BitNet-style cross-attention (8 NeuronCores).

Strategy: pure data-parallel token sharding. b=2, n=2048 -> 4096 query-token
rows; each of the 8 cores owns 512 of them (cores 0-3 batch 0, 4-7 batch 1)
and computes its output slice fully independently.

Engine assignment is driven by measured TRN2 per-op rates:
 - Act (scalar) engine handles every per-partition-pointer scale multiply
   (~1ns/elem) -- DVE's TensorScalarPtr path runs ~14ns/elem.
 - DVE handles immediate-scalar dual-op tensor_scalar, reductions, copies.
 - round() is the exact fp32 magic-constant trick ((x+3*2^22)-3*2^22,
   round-half-even, matches jnp.round) -- no int8 intermediates anywhere
   on the DVE (i8 in/out runs 4x slow there).
 - Activations are quantized in token-major layout (per-token absmax is a
   free-dim reduce; quant scales are per-partition Act pointers), then moved
   to feature-major via XBAR DMA transposes (bf16).
 - Per-tensor weight scales are folded into downstream ops: mWq*mWk/sqrt(D)
   into the Exp activation scale, 1/mWv into the appended ones column of v
   (denominator), mWo into the attn-out dequant scale.
"""

import numpy as np

import concourse.bass as bass
import concourse.mybir as mybir
import concourse.tile as tile
from concourse import bacc, bass_isa
from concourse.bass_utils import run_bass_kernel_spmd

F32 = mybir.dt.float32
BF16 = mybir.dt.bfloat16
I8 = mybir.dt.int8
AX = mybir.AxisListType
OP = mybir.AluOpType
AF = mybir.ActivationFunctionType

P = 128
M_RND = 12582912.0            # 3 * 2^22: fp32 round-to-nearest-even magic

CFG_FULL = dict(DIM=1024, INNER=1024, H=16, D=64, NTOK=512, MCTX=2048)
N_CORES = 8
EPS = 1e-5


def build(cfg):
    DIM, INNER, H, D = cfg["DIM"], cfg["INNER"], cfg["H"], cfg["D"]
    NTOK, MCTX = cfg["NTOK"], cfg["MCTX"]
    KC = DIM // P          # input-dim chunks
    IC = INNER // P        # inner-dim chunks
    NKB = MCTX // P        # key blocks
    NTB = NTOK // P        # query-token 128-blocks
    NCB = MCTX // P        # ctx token 128-blocks
    CW = 512               # k-proj moving width
    CTB = MCTX // CW       # ctx 512-col stripes
    IW = 512               # v/out proj moving width
    NH = INNER // IW
    VW = D + 1             # v columns per head incl ones

    nc = bacc.Bacc("TRN2", target_bir_lowering=False, debug=False,
                   num_devices=N_CORES)

    xn = nc.dram_tensor("xn", [NTOK, DIM], F32, kind="ExternalInput")
    cn = nc.dram_tensor("cn", [MCTX, DIM], F32, kind="ExternalInput")
    wT = {}
    for w in ("wq", "wk", "wv", "wo"):
        wT[w] = nc.dram_tensor(w + "T", [DIM, INNER], F32, kind="ExternalInput")
    y_out = nc.dram_tensor("y", [NTOK, DIM], F32, kind="ExternalOutput")

    from contextlib import ExitStack
    with tile.TileContext(nc) as tc, ExitStack() as ctx:
        pp = ctx.enter_context(tc.tile_pool(name="persist", bufs=1))
        smp = ctx.enter_context(tc.tile_pool(name="small", bufs=1))
        wstp = ctx.enter_context(tc.tile_pool(name="wstage", bufs=1))
        wsc = ctx.enter_context(tc.tile_pool(name="wscratch", bufs=2))
        wbp = ctx.enter_context(tc.tile_pool(name="wbpool", bufs=2))
        ps_proj = ctx.enter_context(tc.tile_pool(name="ps_proj", bufs=2,
                                                 space="PSUM"))
        ps_sc = ctx.enter_context(tc.tile_pool(name="ps_sc", bufs=2,
                                               space="PSUM"))
        ps_o = ctx.enter_context(tc.tile_pool(name="ps_o", bufs=2,
                                              space="PSUM"))

        # ---- persistent SBUF tensors (live across phases) ----------------
        qb = pp.tile([P, IC * NTOK], BF16, tag="qb")      # q, T-major (raw)
        kb = pp.tile([P, IC * MCTX], BF16, tag="kb")      # k, T-major (raw)
        vb = pp.tile([P, NKB * H * VW], BF16, tag="vb")   # v natural + 1/mWv

        wmean = {}

        # ---- weight quantization -----------------------------------------
        # mean|w| via Act Abs + accum_out; ternarize = Act ptr-scale mult,
        # DVE immediate clamp, DVE magic round -> bf16 in {-1,0,+1}.
        def quant_weight(w):
            wst = wstp.tile([P, KC, INNER], F32, tag="wst")
            nc.sync.dma_start(
                out=wst[:],
                in_=wT[w].ap().rearrange("(c p) i -> p c i", p=P))
            asum = smp.tile([P, KC], F32, tag="wasum", name="asum_" + w)
            for c in range(KC):
                nc.vector.tensor_reduce(asum[:, c:c + 1], wst[:, c, :],
                                        axis=AX.X, op=OP.add,
                                        apply_absolute_value=True)
            tsum = smp.tile([P, 1], F32, tag="wtsum")
            nc.vector.tensor_reduce(tsum[:], asum[:], axis=AX.X, op=OP.add)
            wrep = smp.tile([P, 1], F32, tag="wrep")
            nc.gpsimd.partition_all_reduce(wrep[:], tsum[:], channels=P,
                                           reduce_op=bass_isa.ReduceOp.add)
            mean = smp.tile([P, 1], F32, tag="wmean_" + w, name="mean_" + w)
            nc.vector.tensor_scalar(mean[:], wrep[:], 1.0 / (DIM * INNER),
                                    EPS, OP.mult, OP.max)
            qs = smp.tile([P, 1], F32, tag="wqs_" + w, name="qs_" + w)
            nc.vector.reciprocal(qs[:], mean[:])
            wmean[w] = mean
            wbt = wbp.tile([P, KC * INNER], BF16, tag="wb", name="wb_" + w)
            for c in range(KC):
                u = wsc.tile([P, INNER], F32, tag="wu")
                nc.scalar.mul(u[:], wst[:, c, :], qs[:])
                nc.vector.tensor_scalar(u[:], u[:], 1.49, -1.49,
                                        OP.min, OP.max)
                nc.vector.tensor_scalar(wbt[:, c * INNER:(c + 1) * INNER],
                                        u[:], M_RND, -M_RND, OP.add, OP.add)
            return wbt

        # ---- activation quantization (token-major + DMA transpose) -------
        def act_quant(src, dstT, ncols, asp, b0, b1):
            for b in range(b0, b1):
                st = asp.tile([P, DIM], F32, tag="ast")
                nc.sync.dma_start(out=st[:],
                                  in_=src.ap()[b * P:(b + 1) * P, :])
                amax = asp.tile([P, 1], F32, tag="aamax")
                nc.vector.tensor_reduce(amax[:], st[:], axis=AX.X, op=OP.max,
                                        apply_absolute_value=True)
                inv = asp.tile([P, 1], F32, tag="ainv")
                nc.vector.tensor_scalar(inv[:], amax[:], EPS, 1.0 / 127.0,
                                        OP.max, OP.mult)
                qsc = asp.tile([P, 1], F32, tag="aqsc")
                nc.vector.reciprocal(qsc[:], inv[:])
                nc.scalar.mul(st[:], st[:], qsc[:])
                nc.vector.tensor_scalar(st[:], st[:], M_RND, -M_RND,
                                        OP.add, OP.add)
                dq = asp.tile([P, DIM], BF16, tag="adq")
                nc.scalar.mul(dq[:], st[:], inv[:])
                # issue transposes from the Act HWDGE: the SP engine otherwise
                # serializes ~200us of DMA issue on the critical path
                for c in range(KC):
                    nc.scalar.dma_start_transpose(
                        out=dstT[:, c * ncols + b * P: c * ncols + (b + 1) * P],
                        in_=dq[:, c * P:(c + 1) * P])

        with ExitStack() as phase12:
            adp = phase12.enter_context(tc.tile_pool(name="adpool", bufs=1))
            asp = phase12.enter_context(tc.tile_pool(name="astage", bufs=3))
            xdT = adp.tile([P, KC * NTOK], BF16, tag="xdT")
            cdT = adp.tile([P, KC * MCTX], BF16, tag="cdT")

            # wq first so its DMAs lead; x quant overlaps on other engines
            wqb = quant_weight("wq")
            act_quant(xn, xdT, NTOK, asp, 0, NTB)
            for ic in range(IC):
                ps = ps_proj.tile([P, NTOK], F32, tag="pp", name="psq")
                for c in range(KC):
                    nc.tensor.matmul(
                        ps[:],
                        wqb[:, c * INNER + ic * P: c * INNER + (ic + 1) * P],
                        xdT[:, c * NTOK:(c + 1) * NTOK],
                        start=(c == 0), stop=(c == KC - 1))
                nc.vector.tensor_copy(qb[:, ic * NTOK:(ic + 1) * NTOK], ps[:])

            wkb = quant_weight("wk")
            # scores scale mWq*mWk/sqrt(D) -> folded into Exp's scale operand
            qkmul = smp.tile([P, 1], F32, tag="qkmul")
            nc.vector.tensor_tensor(qkmul[:], wmean["wq"][:], wmean["wk"][:],
                                    op=OP.mult)
            qksc = smp.tile([P, 1], F32, tag="qksc")
            nc.vector.tensor_scalar(qksc[:], qkmul[:], 1.0 / np.sqrt(D), None,
                                    OP.mult)
            # ctx quant interleaved with k projection per 512-col stripe
            for tb in range(CTB):
                act_quant(cn, cdT, MCTX, asp,
                          tb * (CW // P), (tb + 1) * (CW // P))
                for ic in range(IC):
                    ps = ps_proj.tile([P, CW], F32, tag="pp", name="psk")
                    for c in range(KC):
                        nc.tensor.matmul(
                            ps[:],
                            wkb[:, c * INNER + ic * P: c * INNER + (ic + 1) * P],
                            cdT[:, c * MCTX + tb * CW: c * MCTX + (tb + 1) * CW],
                            start=(c == 0), stop=(c == KC - 1))
                    nc.vector.tensor_copy(
                        kb[:, ic * MCTX + tb * CW: ic * MCTX + (tb + 1) * CW],
                        ps[:])

            wvb = quant_weight("wv")
            vb3 = vb[:].rearrange("p (k h w) -> p k h w", h=H, w=VW)
            # ones column carries 1/mWv so the denominator folds in v's scale
            rmv = smp.tile([P, 1], F32, tag="rmv")
            nc.vector.reciprocal(rmv[:], wmean["wv"][:])
            nc.vector.memset(vb3[:, :, :, D], 1.0)
            nc.scalar.mul(vb3[:, :, :, D], vb3[:, :, :, D], rmv[:])
            for kbk in range(NKB):
                for ih in range(NH):
                    ps = ps_proj.tile([P, IW], F32, tag="pp", name="psv")
                    for c in range(KC):
                        nc.tensor.matmul(
                            ps[:],
                            cdT[:, c * MCTX + kbk * P: c * MCTX + (kbk + 1) * P],
                            wvb[:, c * INNER + ih * IW: c * INNER + (ih + 1) * IW],
                            start=(c == 0), stop=(c == KC - 1))
                    hph = IW // D
                    nc.vector.tensor_copy(
                        vb3[:, kbk, ih * hph:(ih + 1) * hph, 0:D],
                        ps[:].rearrange("p (h d) -> p h d", d=D))

        # ---- attention ---------------------------------------------------
        # (opool allocated only now, in SBUF space freed by phase12 pools)
        op_pool = ctx.enter_context(tc.tile_pool(name="opool", bufs=1))
        otT = op_pool.tile([P, IC * NTOK], F32, tag="otT")
        oqdT = op_pool.tile([P, IC * NTOK], BF16, tag="oqdT")
        den = op_pool.tile([H, NTOK], F32, tag="den")
        with tc.tile_pool(name="etile", bufs=4) as ep:
            for hp in range(H // 2):
                hA, hB = 2 * hp, 2 * hp + 1
                icA, pA = (hA * D) // P, (hA * D) % P
                icB, pB = (hB * D) // P, (hB * D) % P
                popool, potag = (ps_o, "po") if hp % 2 == 0 else (ps_proj, "pp")
                po = [popool.tile([P, NTOK], F32, tag=potag, name=f"po{hp}_{j}")
                      for j in range(2)]
                for kbk in range(NKB):
                    ss = ps_sc.tile([P, 2, NTOK], F32, tag="ss", name="ss")
                    for j, (h, ich, ph) in enumerate(
                            [(hA, icA, pA), (hB, icB, pB)]):
                        nc.tensor.matmul(
                            ss[:, j, :],
                            kb[ph:ph + D,
                               ich * MCTX + kbk * P: ich * MCTX + (kbk + 1) * P],
                            qb[ph:ph + D, ich * NTOK:(ich + 1) * NTOK],
                            start=True, stop=True)
                    et = ep.tile([P, 2, NTOK], BF16, tag="et")
                    nc.scalar.activation(et[:], ss[:], AF.Exp, scale=qksc[:])
                    for j, h in enumerate((hA, hB)):
                        nc.tensor.matmul(
                            po[j][0:VW, :],
                            vb3[:, kbk, h, :],
                            et[:, j, :],
                            start=(kbk == 0), stop=(kbk == NKB - 1))
                # evict unnormalized; per-head denominators gathered into den
                # (engine APs may only start at partition 0/32/64/96, so the
                # scatter to den row h goes through a small SBUF DMA)
                for j, (h, ich, ph) in enumerate([(hA, icA, pA), (hB, icB, pB)]):
                    nc.vector.tensor_copy(
                        otT[ph:ph + D, ich * NTOK:(ich + 1) * NTOK],
                        po[j][0:D, :])
                    dstage = ep.tile([1, NTOK], F32, tag="dstage")
                    nc.vector.tensor_copy(dstage[:], po[j][D:D + 1, :])
                    nc.sync.dma_start(out=den[h:h + 1, :], in_=dstage[:])

        # one reciprocal for all 16 heads, then broadcast + normalize
        with tc.tile_pool(name="dnp", bufs=2) as dnp:
            rdall = op_pool.tile([H, NTOK], F32, tag="rdall")
            nc.vector.reciprocal(rdall[:], den[:])
            # chunk c holds head 2c on partitions 0-63 and 2c+1 on 64-127;
            # engine APs need matching base partitions, so stage/broadcast
            # each half at its own base and normalize per chunk
            # partition_broadcast only works with base-0 in/out on HW, so
            # broadcast both halves at base 0 and DMA the odd head's half up
            for c in range(IC):
                rstA = dnp.tile([1, NTOK], F32, tag="rstA")
                rstB = dnp.tile([1, NTOK], F32, tag="rstB")
                nc.sync.dma_start(out=rstA[:], in_=rdall[2 * c:2 * c + 1, :])
                nc.sync.dma_start(out=rstB[:],
                                  in_=rdall[2 * c + 1:2 * c + 2, :])
                rb = dnp.tile([P, NTOK], F32, tag="rb")
                rbB = dnp.tile([D, NTOK], F32, tag="rbB")
                nc.gpsimd.partition_broadcast(rb[0:64, :], rstA[:])
                nc.gpsimd.partition_broadcast(rbB[:], rstB[:])
                nc.sync.dma_start(out=rb[64:128, :], in_=rbB[:])
                nc.vector.tensor_tensor(
                    otT[:, c * NTOK:(c + 1) * NTOK],
                    otT[:, c * NTOK:(c + 1) * NTOK],
                    rb[:], op=OP.mult)

        # ---- attn-out quantization + output projection -------------------
        with tc.tile_pool(name="oq", bufs=2) as oqp, \
                tc.tile_pool(name="ysb", bufs=2) as yp:
            wob = quant_weight("wo")
            ot3 = otT[:].rearrange("p (c t) -> p c t", c=IC)
            oamax = oqp.tile([P, NTOK], F32, tag="oamax")
            for c in range(IC):
                arep = oqp.tile([P, NTOK], F32, tag="oarep")
                nc.gpsimd.partition_all_reduce(
                    arep[:], ot3[:, c, :], channels=P,
                    reduce_op=bass_isa.ReduceOp.absmax)
                if c == 0:
                    nc.vector.tensor_copy(oamax[:], arep[:])
                else:
                    nc.vector.tensor_tensor(oamax[:], oamax[:], arep[:],
                                            op=OP.max)
            oinv = oqp.tile([P, NTOK], F32, tag="oinv")
            nc.vector.tensor_scalar(oinv[:], oamax[:], EPS, 1.0 / 127.0,
                                    OP.max, OP.mult)
            oqsc = oqp.tile([P, NTOK], F32, tag="oqsc")
            nc.vector.reciprocal(oqsc[:], oinv[:])
            # fold mWo into the dequant scale so y eviction is a plain copy
            oinv2 = oqp.tile([P, NTOK], F32, tag="oinv2")
            nc.scalar.mul(oinv2[:], oinv[:], wmean["wo"][:])
            for c in range(IC):
                r = oqp.tile([P, NTOK], F32, tag="oqr")
                nc.vector.tensor_tensor(r[:], ot3[:, c, :], oqsc[:], op=OP.mult)
                nc.vector.tensor_scalar(r[:], r[:], M_RND, -M_RND,
                                        OP.add, OP.add)
                nc.vector.tensor_tensor(oqdT[:, c * NTOK:(c + 1) * NTOK],
                                        r[:], oinv2[:], op=OP.mult)

            for tb in range(NTB):
                for oh in range(DIM // IW):
                    ps = ps_proj.tile([P, IW], F32, tag="pp", name="psy")
                    for c in range(IC):
                        nc.tensor.matmul(
                            ps[:],
                            oqdT[:, c * NTOK + tb * P: c * NTOK + (tb + 1) * P],
                            wob[:, c * INNER + oh * IW: c * INNER + (oh + 1) * IW],
                            start=(c == 0), stop=(c == IC - 1))
                    ysb = yp.tile([P, IW], F32, tag="ysb")
                    nc.vector.tensor_copy(ysb[:], ps[:])
                    nc.sync.dma_start(
                        out=y_out.ap()[tb * P:(tb + 1) * P,
                                       oh * IW:(oh + 1) * IW],
                        in_=ysb[:])
    nc.compile()
    return nc


_CACHE = {}


def _get_nc(key, cfg):
    if key not in _CACHE:
        _CACHE[key] = build(cfg)
    return _CACHE[key]


def _shard(x, context, wq, wk, wv, wo, NTOK):
    b = x.shape[0]
    wmaps = {w + "T": np.ascontiguousarray(a.T)
             for w, a in (("wq", wq), ("wk", wk), ("wv", wv), ("wo", wo))}
    cores_per_b = N_CORES // b
    in_maps = []
    for core in range(N_CORES):
        bi = core // cores_per_b
        t0 = (core % cores_per_b) * NTOK
        in_maps.append(dict(
            xn=np.ascontiguousarray(x[bi, t0:t0 + NTOK, :]),
            cn=np.ascontiguousarray(context[bi]),
            **wmaps))
    return in_maps


def _assemble(results, b, n, dim, NTOK):
    out = np.empty((b, n, dim), dtype=np.float32)
    cores_per_b = N_CORES // b
    for core in range(N_CORES):
        bi = core // cores_per_b
        t0 = (core % cores_per_b) * NTOK
        out[bi, t0:t0 + NTOK, :] = results[core]["y"]
    return out


def run(x, context, wq, wk, wv, wo, trace=False):
    cfg = CFG_FULL
    b, n, dim = x.shape
    NTOK = cfg["NTOK"]
    nc = _get_nc("full", cfg)
    in_maps = _shard(x, context, wq, wk, wv, wo, NTOK)
    res = run_bass_kernel_spmd(nc, in_maps, list(range(N_CORES)), trace=trace)
    return _assemble(res.results, b, n, dim, NTOK), res


def kernel(x, context, wq, wk, wv, wo):
    return run(x, context, wq, wk, wv, wo, trace=False)[0]


if __name__ == "__main__":
    ins = {k: np.random.randn(*s).astype(np.float32) * (0.02 if k[0] == 'w' else 1.0)
           for k, s in [("x", (2, 2048, 1024)), ("context", (2, 2048, 1024)),
                        ("wq", (1024, 1024)), ("wk", (1024, 1024)),
                        ("wv", (1024, 1024)), ("wo", (1024, 1024))]}
    y = kernel(**ins)
    print("kernel output", y.shape, y.dtype, np.abs(y).max())


# revision 20
# speedup vs baseline: 1.3858x; 1.3858x over previous
"""Trainium2 Bass kernel for BitNet-style cross-attention (8 NeuronCores).

Strategy: pure data-parallel token sharding. b=2, n=2048 -> 4096 query-token
rows; each of the 8 cores owns 512 of them (cores 0-3 batch 0, 4-7 batch 1)
and computes its output slice fully independently.

Engine assignment is driven by measured TRN2 per-op rates:
 - Act (scalar) engine handles every per-partition-pointer scale multiply
   (~1ns/elem) -- DVE's TensorScalarPtr path runs ~14ns/elem.
 - DVE handles immediate-scalar dual-op tensor_scalar, reductions, copies.
 - round() is the exact fp32 magic-constant trick ((x+3*2^22)-3*2^22,
   round-half-even, matches jnp.round) -- no int8 intermediates anywhere
   on the DVE (i8 in/out runs 4x slow there).
 - Activations are quantized in token-major layout (per-token absmax is a
   free-dim reduce; quant scales are per-partition Act pointers), then moved
   to feature-major via XBAR DMA transposes (bf16).
 - Per-tensor weight scales are folded into downstream ops: mWq*mWk/sqrt(D)
   into the Exp activation scale, 1/mWv into the appended ones column of v
   (denominator), mWo into the attn-out dequant scale.
"""

import numpy as np

import concourse.bass as bass
import concourse.mybir as mybir
import concourse.tile as tile
from concourse import bacc, bass_isa
from concourse.bass_utils import run_bass_kernel_spmd

F32 = mybir.dt.float32
BF16 = mybir.dt.bfloat16
I8 = mybir.dt.int8
AX = mybir.AxisListType
OP = mybir.AluOpType
AF = mybir.ActivationFunctionType

P = 128
M_RND = 12582912.0            # 3 * 2^22: fp32 round-to-nearest-even magic

CFG_FULL = dict(DIM=1024, INNER=1024, H=16, D=64, NTOK=512, MCTX=2048)
N_CORES = 8
EPS = 1e-5


def build(cfg):
    DIM, INNER, H, D = cfg["DIM"], cfg["INNER"], cfg["H"], cfg["D"]
    NTOK, MCTX = cfg["NTOK"], cfg["MCTX"]
    KC = DIM // P          # input-dim chunks
    IC = INNER // P        # inner-dim chunks
    NKB = MCTX // P        # key blocks
    NTB = NTOK // P        # query-token 128-blocks
    NCB = MCTX // P        # ctx token 128-blocks
    CW = 512               # k-proj moving width
    CTB = MCTX // CW       # ctx 512-col stripes
    IW = 512               # v/out proj moving width
    NH = INNER // IW
    VW = D + 1             # v columns per head incl ones

    nc = bacc.Bacc("TRN2", target_bir_lowering=False, debug=False,
                   num_devices=N_CORES)

    xn = nc.dram_tensor("xn", [NTOK, DIM], F32, kind="ExternalInput")
    cn = nc.dram_tensor("cn", [MCTX, DIM], F32, kind="ExternalInput")
    wT = {}
    for w in ("wq", "wk", "wv", "wo"):
        wT[w] = nc.dram_tensor(w + "T", [DIM, INNER], F32, kind="ExternalInput")
    y_out = nc.dram_tensor("y", [NTOK, DIM], F32, kind="ExternalOutput")

    from contextlib import ExitStack
    with tile.TileContext(nc) as tc, ExitStack() as ctx:
        pp = ctx.enter_context(tc.tile_pool(name="persist", bufs=1))
        smp = ctx.enter_context(tc.tile_pool(name="small", bufs=1))
        wstp = ctx.enter_context(tc.tile_pool(name="wstage", bufs=1))
        wsc = ctx.enter_context(tc.tile_pool(name="wscratch", bufs=2))
        wbp = ctx.enter_context(tc.tile_pool(name="wbpool", bufs=2))
        ps_proj = ctx.enter_context(tc.tile_pool(name="ps_proj", bufs=2,
                                                 space="PSUM"))
        ps_sc = ctx.enter_context(tc.tile_pool(name="ps_sc", bufs=2,
                                               space="PSUM"))
        ps_o = ctx.enter_context(tc.tile_pool(name="ps_o", bufs=2,
                                              space="PSUM"))

        # ---- persistent SBUF tensors (live across phases) ----------------
        qb = pp.tile([P, IC * NTOK], BF16, tag="qb")      # q, T-major (raw)
        kb = pp.tile([P, IC * MCTX], BF16, tag="kb")      # k, T-major (raw)
        vb = pp.tile([P, NKB * H * VW], BF16, tag="vb")   # v natural + 1/mWv

        wmean = {}

        # ---- weight quantization -----------------------------------------
        # mean|w| via Act Abs + accum_out; ternarize = Act ptr-scale mult,
        # DVE immediate clamp, DVE magic round -> bf16 in {-1,0,+1}.
        def quant_weight(w):
            wst = wstp.tile([P, KC, INNER], F32, tag="wst")
            for c in range(KC):
                nc.sync.dma_start(out=wst[:, c, :],
                                  in_=wT[w].ap()[c * P:(c + 1) * P, :])
            asum = smp.tile([P, KC], F32, tag="wasum", name="asum_" + w)
            for c in range(KC):
                nc.vector.tensor_reduce(asum[:, c:c + 1], wst[:, c, :],
                                        axis=AX.X, op=OP.add,
                                        apply_absolute_value=True)
            tsum = smp.tile([P, 1], F32, tag="wtsum")
            nc.vector.tensor_reduce(tsum[:], asum[:], axis=AX.X, op=OP.add)
            wrep = smp.tile([P, 1], F32, tag="wrep")
            nc.gpsimd.partition_all_reduce(wrep[:], tsum[:], channels=P,
                                           reduce_op=bass_isa.ReduceOp.add)
            mean = smp.tile([P, 1], F32, tag="wmean_" + w, name="mean_" + w)
            nc.vector.tensor_scalar(mean[:], wrep[:], 1.0 / (DIM * INNER),
                                    EPS, OP.mult, OP.max)
            qs = smp.tile([P, 1], F32, tag="wqs_" + w, name="qs_" + w)
            nc.vector.reciprocal(qs[:], mean[:])
            wmean[w] = mean
            wbt = wbp.tile([P, KC * INNER], BF16, tag="wb", name="wb_" + w)
            for c in range(KC):
                u = wsc.tile([P, INNER], F32, tag="wu")
                nc.scalar.mul(u[:], wst[:, c, :], qs[:])
                nc.vector.tensor_scalar(u[:], u[:], 1.49, -1.49,
                                        OP.min, OP.max)
                nc.vector.tensor_scalar(wbt[:, c * INNER:(c + 1) * INNER],
                                        u[:], M_RND, -M_RND, OP.add, OP.add)
            return wbt

        # ---- activation quantization (token-major + DMA transpose) -------
        # dstT4 layout is block-major [P, blocks, KC, P] so each 128-token
        # block needs ONE XBAR transpose (3D contiguous destination) --
        # HWDGE issue costs ~1.2us of issuing-engine time per DMA
        def act_quant(src, dstT4, asp, b0, b1):
            for b in range(b0, b1):
                st = asp.tile([P, DIM], F32, tag="ast")
                nc.sync.dma_start(out=st[:],
                                  in_=src.ap()[b * P:(b + 1) * P, :])
                amax = asp.tile([P, 1], F32, tag="aamax")
                nc.vector.tensor_reduce(amax[:], st[:], axis=AX.X, op=OP.max,
                                        apply_absolute_value=True)
                inv = asp.tile([P, 1], F32, tag="ainv")
                nc.vector.tensor_scalar(inv[:], amax[:], EPS, 1.0 / 127.0,
                                        OP.max, OP.mult)
                qsc = asp.tile([P, 1], F32, tag="aqsc")
                nc.vector.reciprocal(qsc[:], inv[:])
                nc.scalar.mul(st[:], st[:], qsc[:])
                nc.vector.tensor_scalar(st[:], st[:], M_RND, -M_RND,
                                        OP.add, OP.add)
                dq = asp.tile([P, DIM], BF16, tag="adq")
                nc.scalar.mul(dq[:], st[:], inv[:])
                nc.sync.dma_start_transpose(out=dstT4[:, b, :, :], in_=dq[:])

        with ExitStack() as phase12:
            adp = phase12.enter_context(tc.tile_pool(name="adpool", bufs=1))
            asp = phase12.enter_context(tc.tile_pool(name="astage", bufs=3))
            xdT = adp.tile([P, NTB, KC, P], BF16, tag="xdT")
            cdT = adp.tile([P, NCB, KC, P], BF16, tag="cdT")

            # wq first so its DMAs lead; x quant overlaps on other engines
            wqb = quant_weight("wq")
            act_quant(xn, xdT, asp, 0, NTB)
            for ic in range(IC):
                ps = ps_proj.tile([P, NTOK], F32, tag="pp", name="psq")
                for c in range(KC):
                    nc.tensor.matmul(
                        ps[:],
                        wqb[:, c * INNER + ic * P: c * INNER + (ic + 1) * P],
                        xdT[:, :, c, :],
                        start=(c == 0), stop=(c == KC - 1))
                nc.vector.tensor_copy(qb[:, ic * NTOK:(ic + 1) * NTOK], ps[:])

            wkb = quant_weight("wk")
            # scores scale mWq*mWk/sqrt(D) -> folded into Exp's scale operand
            qkmul = smp.tile([P, 1], F32, tag="qkmul")
            nc.vector.tensor_tensor(qkmul[:], wmean["wq"][:], wmean["wk"][:],
                                    op=OP.mult)
            qksc = smp.tile([P, 1], F32, tag="qksc")
            nc.vector.tensor_scalar(qksc[:], qkmul[:], 1.0 / np.sqrt(D), None,
                                    OP.mult)
            # ctx quant interleaved with k projection per 512-col stripe
            for tb in range(CTB):
                act_quant(cn, cdT, asp,
                          tb * (CW // P), (tb + 1) * (CW // P))
                nbk = CW // P
                for ic in range(IC):
                    ps = ps_proj.tile([P, CW], F32, tag="pp", name="psk")
                    for c in range(KC):
                        nc.tensor.matmul(
                            ps[:],
                            wkb[:, c * INNER + ic * P: c * INNER + (ic + 1) * P],
                            cdT[:, tb * nbk:(tb + 1) * nbk, c, :],
                            start=(c == 0), stop=(c == KC - 1))
                    nc.vector.tensor_copy(
                        kb[:, ic * MCTX + tb * CW: ic * MCTX + (tb + 1) * CW],
                        ps[:])

            wvb = quant_weight("wv")
            vb3 = vb[:].rearrange("p (k h w) -> p k h w", h=H, w=VW)
            # ones column carries 1/mWv so the denominator folds in v's scale
            rmv = smp.tile([P, 1], F32, tag="rmv")
            nc.vector.reciprocal(rmv[:], wmean["wv"][:])
            nc.vector.memset(vb3[:, :, :, D], 1.0)
            nc.scalar.mul(vb3[:, :, :, D], vb3[:, :, :, D], rmv[:])
            for kbk in range(NKB):
                for ih in range(NH):
                    ps = ps_proj.tile([P, IW], F32, tag="pp", name="psv")
                    for c in range(KC):
                        nc.tensor.matmul(
                            ps[:],
                            cdT[:, kbk, c, :],
                            wvb[:, c * INNER + ih * IW: c * INNER + (ih + 1) * IW],
                            start=(c == 0), stop=(c == KC - 1))
                    hph = IW // D
                    nc.vector.tensor_copy(
                        vb3[:, kbk, ih * hph:(ih + 1) * hph, 0:D],
                        ps[:].rearrange("p (h d) -> p h d", d=D))

        # ---- attention ---------------------------------------------------
        # (opool allocated only now, in SBUF space freed by phase12 pools)
        op_pool = ctx.enter_context(tc.tile_pool(name="opool", bufs=1))
        otT = op_pool.tile([P, IC * NTOK], F32, tag="otT")
        oqdT = op_pool.tile([P, IC * NTOK], BF16, tag="oqdT")
        den = op_pool.tile([H, NTOK], F32, tag="den")
        with tc.tile_pool(name="etile", bufs=4) as ep:
            for hp in range(H // 2):
                hA, hB = 2 * hp, 2 * hp + 1
                icA, pA = (hA * D) // P, (hA * D) % P
                icB, pB = (hB * D) // P, (hB * D) % P
                popool, potag = (ps_o, "po") if hp % 2 == 0 else (ps_proj, "pp")
                po = [popool.tile([P, NTOK], F32, tag=potag, name=f"po{hp}_{j}")
                      for j in range(2)]
                for kbk in range(NKB):
                    ss = ps_sc.tile([P, 2, NTOK], F32, tag="ss", name="ss")
                    for j, (h, ich, ph) in enumerate(
                            [(hA, icA, pA), (hB, icB, pB)]):
                        nc.tensor.matmul(
                            ss[:, j, :],
                            kb[ph:ph + D,
                               ich * MCTX + kbk * P: ich * MCTX + (kbk + 1) * P],
                            qb[ph:ph + D, ich * NTOK:(ich + 1) * NTOK],
                            start=True, stop=True)
                    et = ep.tile([P, 2, NTOK], BF16, tag="et")
                    nc.scalar.activation(et[:], ss[:], AF.Exp, scale=qksc[:])
                    for j, h in enumerate((hA, hB)):
                        nc.tensor.matmul(
                            po[j][0:VW, :],
                            vb3[:, kbk, h, :],
                            et[:, j, :],
                            start=(kbk == 0), stop=(kbk == NKB - 1))
                # evict unnormalized; per-head denominators gathered into den
                # (engine APs may only start at partition 0/32/64/96, so the
                # scatter to den row h goes through a small SBUF DMA)
                for j, (h, ich, ph) in enumerate([(hA, icA, pA), (hB, icB, pB)]):
                    nc.vector.tensor_copy(
                        otT[ph:ph + D, ich * NTOK:(ich + 1) * NTOK],
                        po[j][0:D, :])
                    dstage = ep.tile([1, NTOK], F32, tag="dstage")
                    nc.vector.tensor_copy(dstage[:], po[j][D:D + 1, :])
                    nc.sync.dma_start(out=den[h:h + 1, :], in_=dstage[:])

        # one reciprocal for all 16 heads, then broadcast + normalize
        with tc.tile_pool(name="dnp", bufs=2) as dnp:
            rdall = op_pool.tile([H, NTOK], F32, tag="rdall")
            nc.vector.reciprocal(rdall[:], den[:])
            # chunk c holds head 2c on partitions 0-63 and 2c+1 on 64-127;
            # engine APs need matching base partitions, so stage/broadcast
            # each half at its own base and normalize per chunk
            # partition_broadcast only works with base-0 in/out on HW, so
            # broadcast both halves at base 0 and DMA the odd head's half up
            for c in range(IC):
                rstA = dnp.tile([1, NTOK], F32, tag="rstA")
                rstB = dnp.tile([1, NTOK], F32, tag="rstB")
                nc.sync.dma_start(out=rstA[:], in_=rdall[2 * c:2 * c + 1, :])
                nc.sync.dma_start(out=rstB[:],
                                  in_=rdall[2 * c + 1:2 * c + 2, :])
                rb = dnp.tile([P, NTOK], F32, tag="rb")
                rbB = dnp.tile([D, NTOK], F32, tag="rbB")
                nc.gpsimd.partition_broadcast(rb[0:64, :], rstA[:])
                nc.gpsimd.partition_broadcast(rbB[:], rstB[:])
                nc.sync.dma_start(out=rb[64:128, :], in_=rbB[:])
                nc.vector.tensor_tensor(
                    otT[:, c * NTOK:(c + 1) * NTOK],
                    otT[:, c * NTOK:(c + 1) * NTOK],
                    rb[:], op=OP.mult)

        # ---- attn-out quantization + output projection -------------------
        with tc.tile_pool(name="oq", bufs=2) as oqp, \
                tc.tile_pool(name="ysb", bufs=2) as yp:
            wob = quant_weight("wo")
            ot3 = otT[:].rearrange("p (c t) -> p c t", c=IC)
            oamax = oqp.tile([P, NTOK], F32, tag="oamax")
            for c in range(IC):
                arep = oqp.tile([P, NTOK], F32, tag="oarep")
                nc.gpsimd.partition_all_reduce(
                    arep[:], ot3[:, c, :], channels=P,
                    reduce_op=bass_isa.ReduceOp.absmax)
                if c == 0:
                    nc.vector.tensor_copy(oamax[:], arep[:])
                else:
                    nc.vector.tensor_tensor(oamax[:], oamax[:], arep[:],
                                            op=OP.max)
            oinv = oqp.tile([P, NTOK], F32, tag="oinv")
            nc.vector.tensor_scalar(oinv[:], oamax[:], EPS, 1.0 / 127.0,
                                    OP.max, OP.mult)
            oqsc = oqp.tile([P, NTOK], F32, tag="oqsc")
            nc.vector.reciprocal(oqsc[:], oinv[:])
            # fold mWo into the dequant scale so y eviction is a plain copy
            oinv2 = oqp.tile([P, NTOK], F32, tag="oinv2")
            nc.scalar.mul(oinv2[:], oinv[:], wmean["wo"][:])
            for c in range(IC):
                r = oqp.tile([P, NTOK], F32, tag="oqr")
                nc.vector.tensor_tensor(r[:], ot3[:, c, :], oqsc[:], op=OP.mult)
                nc.vector.tensor_scalar(r[:], r[:], M_RND, -M_RND,
                                        OP.add, OP.add)
                nc.vector.tensor_tensor(oqdT[:, c * NTOK:(c + 1) * NTOK],
                                        r[:], oinv2[:], op=OP.mult)

            for tb in range(NTB):
                for oh in range(DIM // IW):
                    ps = ps_proj.tile([P, IW], F32, tag="pp", name="psy")
                    for c in range(IC):
                        nc.tensor.matmul(
                            ps[:],
                            oqdT[:, c * NTOK + tb * P: c * NTOK + (tb + 1) * P],
                            wob[:, c * INNER + oh * IW: c * INNER + (oh + 1) * IW],
                            start=(c == 0), stop=(c == IC - 1))
                    ysb = yp.tile([P, IW], F32, tag="ysb")
                    nc.vector.tensor_copy(ysb[:], ps[:])
                    nc.sync.dma_start(
                        out=y_out.ap()[tb * P:(tb + 1) * P,
                                       oh * IW:(oh + 1) * IW],
                        in_=ysb[:])
    nc.compile()
    return nc


_CACHE = {}


def _get_nc(key, cfg):
    if key not in _CACHE:
        _CACHE[key] = build(cfg)
    return _CACHE[key]


def _shard(x, context, wq, wk, wv, wo, NTOK):
    b = x.shape[0]
    wmaps = {w + "T": np.ascontiguousarray(a.T)
             for w, a in (("wq", wq), ("wk", wk), ("wv", wv), ("wo", wo))}
    cores_per_b = N_CORES // b
    in_maps = []
    for core in range(N_CORES):
        bi = core // cores_per_b
        t0 = (core % cores_per_b) * NTOK
        in_maps.append(dict(
            xn=np.ascontiguousarray(x[bi, t0:t0 + NTOK, :]),
            cn=np.ascontiguousarray(context[bi]),
            **wmaps))
    return in_maps


def _assemble(results, b, n, dim, NTOK):
    out = np.empty((b, n, dim), dtype=np.float32)
    cores_per_b = N_CORES // b
    for core in range(N_CORES):
        bi = core // cores_per_b
        t0 = (core % cores_per_b) * NTOK
        out[bi, t0:t0 + NTOK, :] = results[core]["y"]
    return out


def run(x, context, wq, wk, wv, wo, trace=False):
    cfg = CFG_FULL
    b, n, dim = x.shape
    NTOK = cfg["NTOK"]
    nc = _get_nc("full", cfg)
    in_maps = _shard(x, context, wq, wk, wv, wo, NTOK)
    res = run_bass_kernel_spmd(nc, in_maps, list(range(N_CORES)), trace=trace)
    return _assemble(res.results, b, n, dim, NTOK), res


def kernel(x, context, wq, wk, wv, wo):
    return run(x, context, wq, wk, wv, wo, trace=False)[0]


if __name__ == "__main__":
    ins = {k: np.random.randn(*s).astype(np.float32) * (0.02 if k[0] == 'w' else 1.0)
           for k, s in [("x", (2, 2048, 1024)), ("context", (2, 2048, 1024)),
                        ("wq", (1024, 1024)), ("wk", (1024, 1024)),
                        ("wv", (1024, 1024)), ("wo", (1024, 1024))]}
    y = kernel(**ins)
    print("kernel output", y.shape, y.dtype, np.abs(y).max())
